# revision 38
# baseline (speedup 1.0000x reference)
"""Trainium2 Bass kernel for nn_DynamicMLP (3-layer LIF spiking net, T=16).

Strategy (8 NeuronCores, data-parallel over batch):
  - Shard batch 1024 -> 8 x 128. Replicate weights. Zero cross-core comms.
  - Layout: [batch=128 partitions, hidden on free dim].
  - The LIF current state c lives ENTIRELY in PSUM, scaled by 2^t:
      C_t = sum_{tau<=t} 2^tau * I_tau  ==  2^t * c_t  (bitwise-equivalent to the
      reference's c = 0.5*c + I decay, since powers of 2 are exact).
    Inputs are pre-scaled by 2^t on host (x) / on device (spikes).
  - The output is chaotically sensitive (1e-6 current noise -> 2% output
    error), so matmuls must be fp32-exact. They run as fp16 multi-term splits
    (fp16 x fp16 products are exact in fp32 PSUM accumulation; all stored
    operands kept in fp16 normal range; ~1e-7 residual):
      L0: x = xh + xl exactly (fp16 pair). 2^t*xh@wh -> C0;
          2^(t+11)*xl@wh and 2^t*xh@(wl*2^11) -> C0b (folded at 2^-(t+11)).
      L1/L2: spikes s*2^t are fp16-exact; s_hi@wh -> C and
          (s_hi*2^-11)@(wl*2^11) -> C, same scale, no extra banks.
    Residual error ~2e-8 per current, inside the fp32 matmul-order envelope.
  - Spikes are emitted as fp16 * 2^t and DMA-transposed (xbar) to become the
    next layer's stationary operand. Biases enter via a K=2 matmul row pair
    (rows scaled 2^t and 2^(t-11) for the hi/lo bias split).
"""
import sys

sys.path.insert(0, "/opt/trn_rl_repo")

import numpy as np

import concourse.bacc as bacc
import concourse.tile as tile
from concourse import mybir
from concourse.bass_utils import run_bass_kernel_spmd

dt = mybir.dt
F16 = dt.float16
F32 = dt.float32
Alu = mybir.AluOpType

NCORES = 8
FULL = dict(T=16, IN=2048, H0=1024, H1=1024, OUT=512, BL=128)
EXACT_ORDER = True  # reproduce the reference LIF rounding order exactly

_BUILD_CACHE = {}


def build(T=16, IN=2048, H0=1024, H1=1024, OUT=512, BL=128):
    key = (T, IN, H0, H1, OUT, BL, EXACT_ORDER)
    if key in _BUILD_CACHE:
        return _BUILD_CACHE[key]
    KT0, KT1, KT2 = IN // 128, H0 // 128, H1 // 128
    NCH = 512  # psum bank free-dim (fp32)

    nc = bacc.Bacc("TRN2", target_bir_lowering=False, debug=False, num_devices=NCORES)

    xa_d = nc.dram_tensor("xa", [T, IN, BL], F16, kind="ExternalInput")
    xr_d = nc.dram_tensor("xr", [T, IN, BL], F16, kind="ExternalInput")
    w_d = {}
    for nm, (a, b) in {"w0": (IN, H0), "w1": (H0, H1), "wo": (H1, OUT)}.items():
        w_d[nm + "a"] = nc.dram_tensor(nm + "a", [a, b], F16, kind="ExternalInput")
        w_d[nm + "l"] = nc.dram_tensor(nm + "l", [a, b], F16, kind="ExternalInput")
    b_d = {}
    for nm, h in {"b0": H0, "b1": H1, "b2": OUT}.items():
        b_d[nm] = nc.dram_tensor(nm, [2, h], F16, kind="ExternalInput")
    ones_d = nc.dram_tensor("onesrows", [2, T * 128], F16, kind="ExternalInput")
    id_d = nc.dram_tensor("ident", [128, 128], F16, kind="ExternalInput")
    out_d = nc.dram_tensor("out", [BL, OUT], F32, kind="ExternalOutput")

    with tile.TileContext(nc) as tc:
        with tc.tile_pool(name="w", bufs=1) as wp, \
             tc.tile_pool(name="state", bufs=1) as sp, \
             tc.tile_pool(name="xs", bufs=3) as xp, \
             tc.tile_pool(name="spk", bufs=2) as kp, \
             tc.tile_pool(name="psum", bufs=1, space="PSUM") as pp:

            # ---- resident weights (DMA order = first-use order) ----
            KH = max(KT0 // 2, 1)
            NX0 = KT0 // KH
            w_sb = {}
            for nm, (kt, h) in {"w1": (KT1, H1), "wo": (KT2, OUT)}.items():
                for sfx in ("a", "l"):
                    w_sb[nm + sfx] = wp.tile([128, kt * h], F16, tag=nm + sfx,
                                             name=nm + sfx)
            # w0 in per-chunk tiles so L0 can start after the first chunk lands
            for sfx in ("a", "l"):
                w_sb["w0" + sfx] = [
                    wp.tile([128, KH * H0], F16, tag=f"w0{sfx}{ci}", name=f"w0{sfx}{ci}")
                    for ci in range(NX0)]

            def dma_weights(nm, kt, h):
                for sfx in ("a", "l"):
                    tl = w_sb[nm + sfx]
                    for k in range(kt):
                        if nm == "w0":
                            nc.sync.dma_start(
                                out=tl[k // KH][:, (k % KH) * h:(k % KH + 1) * h],
                                in_=w_d[nm + sfx][k * 128:(k + 1) * 128, :])
                        else:
                            nc.sync.dma_start(out=tl[:, k * h:(k + 1) * h],
                                              in_=w_d[nm + sfx][k * 128:(k + 1) * 128, :])

            b_sb = {}
            for nm, h in {"b0": H0, "b1": H1, "b2": OUT}.items():
                tl = wp.tile([2, h], F16, tag=nm, name=nm)
                nc.sync.dma_start(out=tl[:], in_=b_d[nm][:])
                b_sb[nm] = tl


            # ---- states (single-buffered; DVE program order serializes) ----
            HS = {0: H0, 1: H1, 2: OUT}
            st = {}
            for l in (0, 1, 2):
                for nm in ("v", "u0", "v0", "q"):
                    st[(l, nm)] = sp.tile([128, HS[l]], F32, tag=f"{nm}{l}", name=f"{nm}{l}")
            c021 = sp.tile([128, max(H0, H1)], F32, tag="c021")
            scrA = sp.tile([128, max(H0, H1)], F32, tag="scrA")
            scrB12 = sp.tile([128, H1], F32, tag="scrB12", name="scrB12")
            scrB0b = sp.tile([128, H0], F32, tag="scrB0b", name="scrB0b")
            scrB = {0: sp.tile([128, H0], F32, tag="scrB0", name="scrB0"),
                    1: scrB12, 2: scrB12}
            # psum current accumulators (2^t-scaled)
            C = {0: pp.tile([128, H0], F32, tag="C0", name="C0"),
                 1: pp.tile([128, H1], F32, tag="C1", name="C1"),
                 2: pp.tile([128, OUT], F32, tag="C2", name="C2")}
            C0b = pp.tile([128, H0], F32, tag="C0b", name="C0b")
            accP = pp.tile([128, OUT], F32, tag="accP", name="accP")
            ident = wp.tile([128, 128], F16, tag="ident", name="ident")
            nc.sync.dma_start(out=ident[:], in_=id_d[:])


            # ---- init ----
            for l in (0, 1, 2):
                for nm in ("v", "u0", "v0", "q"):
                    nc.vector.memset(st[(l, nm)][:], 0.0)
            nc.vector.memset(c021[:], 0.021)

            def lif_B(l, t):
                """Release C[l] (+C0b) into scratch on ACT (short queue, and
                the 2^-t scales are exact powers of two -> no rounding)."""
                h = HS[l]
                nc.scalar.mul(scrB[l][:, :h], C[l][:], float(2.0 ** -t))
                if l == 0:
                    nc.scalar.mul(scrB0b[:], C0b[:], float(2.0 ** -(t + 11)))

            def lif_ops(l, t, s_out, last=False):
                """Emit LIF elementwise ops for layer l at step t.

                Consumes C[l] (psum, = 2^t * c_t), states v0/u0/q from step t-1.
                Produces v (=v_t), updates u0/v0/q for t+1, and (if s_out) the
                2^t-scaled fp16 spike tensor.
                """
                h = HS[l]
                v, u0, v0, q = (st[(l, n)] for n in ("v", "u0", "v0", "q"))
                A = scrA[:, :h]
                if EXACT_ORDER:
                    B = scrB[l][:, :h]
                    if not last:
                        # u_t = u0 + ((-0.172*v0) + 0.529*u0)  (reference rounding)
                        nc.scalar.mul(A, v0[:], -0.172)
                        nc.vector.scalar_tensor_tensor(
                            out=A, in0=u0[:], scalar=0.529, in1=A,
                            op0=Alu.mult, op1=Alu.add)
                        nc.vector.tensor_tensor(out=A, in0=u0[:], in1=A, op=Alu.add)
                    # dv = ((q - v0) - u0) + c;  v = v0 + dv   (reference rounding)
                    nc.vector.tensor_tensor(out=v[:], in0=q[:], in1=v0[:],
                                            op=Alu.subtract)
                    nc.vector.tensor_tensor(out=v[:], in0=v[:], in1=u0[:],
                                            op=Alu.subtract)
                    if l == 0:
                        nc.vector.tensor_tensor(out=v[:], in0=v[:], in1=scrB0b[:],
                                                op=Alu.add)
                    nc.vector.tensor_tensor(out=v[:], in0=v[:], in1=B, op=Alu.add)
                    nc.vector.tensor_tensor(out=v[:], in0=v0[:], in1=v[:],
                                            op=Alu.add)
                else:
                    # u_t = 1.529*(u0 - (0.172/1.529)*v0)   (A := u_t)
                    nc.vector.scalar_tensor_tensor(
                        out=A, in0=v0[:], scalar=float(-0.172 / 1.529), in1=u0[:],
                        op0=Alu.mult, op1=Alu.add)
                    nc.vector.tensor_scalar(out=A, in0=A, scalar1=1.529,
                                            scalar2=None, op0=Alu.mult)
                    # v_t = (q - u0) + [2^-(t+11) * C0b] + 2^-t * C
                    nc.vector.tensor_tensor(out=v[:], in0=q[:], in1=u0[:],
                                            op=Alu.subtract)
                    if l == 0:
                        nc.vector.scalar_tensor_tensor(
                            out=v[:], in0=C0b[:], scalar=float(2.0 ** -(t + 11)),
                            in1=v[:], op0=Alu.mult, op1=Alu.add)
                    nc.vector.scalar_tensor_tensor(
                        out=v[:], in0=C[l][:], scalar=float(2.0 ** -t), in1=v[:],
                        op0=Alu.mult, op1=Alu.add)
                # spikes (scale 2^t for l<2; unscaled for l==2) -> fp16
                s_scale = 1.0 if l == 2 else float(2.0 ** t)
                nc.vector.tensor_scalar(out=s_out, in0=v[:], scalar1=0.5,
                                        scalar2=s_scale, op0=Alu.is_gt,
                                        op1=Alu.mult)
                if l == 2:
                    pending_acc.append((t, s_out))
                if last:
                    return
                # u0_{t+1} = u_t + 0.132 * s_t     (unscale s_out)
                nc.vector.scalar_tensor_tensor(
                    out=u0[:], in0=s_out, scalar=float(0.132 / s_scale), in1=A,
                    op0=Alu.mult, op1=Alu.add)
                # v0_{t+1} = v_t with 0.021 where spiked
                nc.scalar.copy(v0[:], v[:])
                nc.vector.copy_predicated(out=v0[:], mask=s_out.bitcast(dt.uint16),
                                          data=c021[:, :h])
                # q_{t+1} = v0^2
                nc.scalar.square(q[:], v0[:])

            def matmuls(l, t, kt, h, lhsA, lhsR, wa, wl, bias, ones2,
                        k_base=0, bias_too=True, kt_total=None):
                """Accumulate 2^t * (x@W + b) into C[l] (+C0b lo-part for l=0).

                l==0: lhsA = 2^t*xh tiles, lhsR = 2^(t+11)*xl tiles.
                      lhsA@wa -> C0; lhsR@wa -> C0b; lhsA@wl(*2^11) -> C0b.
                l>0:  lhsA = 2^t*s_hi tiles, lhsR = 2^(t-11)*s_hi tiles.
                      lhsA@wa -> C; lhsR@wl(*2^11) -> C.
                start=True is emitted per PSUM bank (each n0 chunk) at t==0.
                """
                kt_total = kt_total if kt_total is not None else kt
                for k in range(kt):
                    kg = k_base + k
                    for n0 in range(0, h, NCH):
                        nn = min(NCH, h - n0)
                        first = (t == 0 and kg == 0)
                        last = (t == T - 1 and kg == kt_total - 1)
                        ps = C[l][:, n0:n0 + nn]
                        ra = wa[:, k * h + n0: k * h + n0 + nn]
                        rl = wl[:, k * h + n0: k * h + n0 + nn]
                        la = lhsA[:, k * 128:(k + 1) * 128]
                        lr = lhsR[:, k * 128:(k + 1) * 128]
                        nc.tensor.matmul(ps, la, ra, start=first,
                                         stop=False, skip_group_check=True)
                        if l == 0:
                            psb = C0b[:, n0:n0 + nn]
                            nc.tensor.matmul(psb, lr, ra, start=first,
                                             stop=False, skip_group_check=True)
                            nc.tensor.matmul(psb, la, rl, start=False, stop=last,
                                             skip_group_check=True)
                        else:
                            nc.tensor.matmul(ps, lr, rl, start=False, stop=False,
                                             skip_group_check=True)
                if bias_too:
                    for n0 in range(0, h, NCH):
                        nn = min(NCH, h - n0)
                        nc.tensor.matmul(C[l][:, n0:n0 + nn], ones2[:],
                                         bias[:, n0:n0 + nn], start=False,
                                         stop=(t == T - 1), skip_group_check=True)

            ones2_h = {}
            pending_acc = []

            def flush_acc():
                while pending_acc:
                    ta, s2ap = pending_acc.pop(0)
                    nc.tensor.matmul(accP[:], ident[:], s2ap, start=(ta == 0),
                                     stop=(ta == T - 1), skip_group_check=True)

            x_pre = {}

            def load_x(t):
                ones2 = xp.tile([2, 128], F16, tag="ones2", name=f"ones2_t{t}")
                nc.sync.dma_start(out=ones2[:], in_=ones_d[:, t * 128:(t + 1) * 128])
                ones2_h[t] = ones2
                tiles = []
                for ci in range(NX0):
                    xa_t = xp.tile([128, KH * BL], F16, tag="xa", name=f"xa_t{t}_{ci}")
                    xr_t = xp.tile([128, KH * BL], F16, tag="xr", name=f"xr_t{t}_{ci}")
                    ks = ci * KH * 128
                    nc.sync.dma_start(
                        out=xa_t[:].rearrange("p (k b) -> p k b", b=BL),
                        in_=xa_d[t:t + 1, ks:ks + KH * 128].rearrange(
                            "o (k p) b -> p (o k) b", p=128))
                    nc.sync.dma_start(
                        out=xr_t[:].rearrange("p (k b) -> p k b", b=BL),
                        in_=xr_d[t:t + 1, ks:ks + KH * 128].rearrange(
                            "o (k p) b -> p (o k) b", p=128))
                    tiles.append((xa_t, xr_t))
                x_pre[t] = tiles

            def emit_L0(t, cis=None):
                if t not in x_pre:
                    load_x(t)
                tiles = x_pre[t]
                if cis is None or 1 in cis:
                    x_pre.pop(t, None)
                ones2 = ones2_h[t]
                for ci in (cis if cis is not None else range(NX0)):
                    xa_t, xr_t = tiles[ci]
                    matmuls(0, t, KH, H0, xa_t[:], xr_t[:],
                            w_sb["w0a"][ci][:], w_sb["w0l"][ci][:],
                            b_sb["b0"], ones2[:], k_base=ci * KH,
                            bias_too=(ci == NX0 - 1), kt_total=KT0)

            def emit_rest(t, filler=None):
                flush_acc()
                ones2 = ones2_h[t]
                s0 = kp.tile([128, H0], F16, tag="sPre", name=f"s0_t{t}")
                lif_ops(0, t, s0[:], last=(t == T - 1))  # B0 emitted by caller
                s0T = kp.tile([128, H0], F16, tag="sT", name=f"s0T_t{t}")
                nc.sync.dma_start_transpose(
                    out=s0T[:].rearrange("p (k b) -> p k b", b=128), in_=s0[:])
                s0L = kp.tile([128, H0], F16, tag="sL", name=f"s0L_t{t}", bufs=2)
                nc.vector.tensor_scalar(out=s0L[:], in0=s0T[:],
                                        scalar1=float(2.0 ** -11), scalar2=None,
                                        op0=Alu.mult)
                matmuls(1, t, KT1, H1, s0T[:], s0L[:], w_sb["w1a"], w_sb["w1l"],
                        b_sb["b1"], ones2[:])
                lif_B(1, t)
                if filler is not None:
                    filler()
                s1 = kp.tile([128, H1], F16, tag="sPre", name=f"s1_t{t}")
                lif_ops(1, t, s1[:], last=(t == T - 1))
                s1T = kp.tile([128, H1], F16, tag="sT", name=f"s1T_t{t}")
                nc.sync.dma_start_transpose(
                    out=s1T[:].rearrange("p (k b) -> p k b", b=128), in_=s1[:])
                s1L = kp.tile([128, H1], F16, tag="sL", name=f"s1L_t{t}", bufs=2)
                nc.vector.tensor_scalar(out=s1L[:], in0=s1T[:],
                                        scalar1=float(2.0 ** -11), scalar2=None,
                                        op0=Alu.mult)
                matmuls(2, t, KT2, OUT, s1T[:], s1L[:], w_sb["woa"], w_sb["wol"],
                        b_sb["b2"], ones2[:])
                lif_B(2, t)
                s2 = kp.tile([128, OUT], F16, tag="s2", name=f"s2_t{t}", bufs=2)
                lif_ops(2, t, s2[:], last=(t == T - 1))
                ones2_h.pop(t, None)

            # preamble DMAs in first-use order: x(0) first, then weights
            load_x(0)
            for ci in range(NX0):
                for sfx in ("a", "l"):
                    tl = w_sb["w0" + sfx][ci]
                    for kk in range(KH):
                        k = ci * KH + kk
                        nc.sync.dma_start(out=tl[:, kk * H0:(kk + 1) * H0],
                                          in_=w_d["w0" + sfx][k * 128:(k + 1) * 128, :])
            dma_weights("w1", KT1, H1)
            dma_weights("wo", KT2, OUT)

            # 1-step layer skew: PE gets L0(t+1) while the t chain drains
            for t in range(T):
                if t >= 1:
                    lif_B(0, t - 1)       # free C0/C0b for step t's matmuls
                emit_L0(t, cis=(0,))
                if t + 1 < T:
                    load_x(t + 1)
                if t >= 1:
                    emit_rest(t - 1, filler=lambda tt=t: emit_L0(tt, cis=(1,)))
                else:
                    emit_L0(t, cis=(1,))
            lif_B(0, T - 1)
            emit_rest(T - 1)

            flush_acc()
            accS = sp.tile([128, OUT], F32, tag="accS", name="accS")
            nc.vector.tensor_copy(out=accS[:], in_=accP[:])
            nc.sync.dma_start(out=out_d[:], in_=accS[:])

    nc.compile()
    _BUILD_CACHE[key] = nc
    return nc


def _split_f16(a32, lo_scale=2048.0):
    """a32 ~ hi + lo*2^-11 with hi = fp16(a32), lo = fp16((a32-hi)*2^11)."""
    hi = a32.astype(np.float16)
    lo = ((a32 - hi.astype(np.float32)) * np.float32(lo_scale)).astype(np.float16)
    return hi, lo


def prep_inputs(in_pop_spikes, W0, b0, W1, b1, Wout, bout,
                T=16, BL=128, ncores=NCORES):
    """Host-side prep: transpose/scale/split x, split weights; 8 in_maps."""
    x = np.ascontiguousarray(np.transpose(np.asarray(in_pop_spikes, np.float32),
                                          (2, 1, 0)))  # [T, IN, B]
    scale = (2.0 ** np.arange(T, dtype=np.float32)).reshape(T, 1, 1)
    xh32 = x.astype(np.float16).astype(np.float32)
    xa = (xh32 * scale).astype(np.float16)                 # exact 2^t * fp16(x)
    xr = ((x - xh32) * (scale * np.float32(2048.0))).astype(np.float16)
    # ^ 2^(t+11) * xl, fp16 (xl itself is the exact fp32 residual)

    com = {}
    for nm, W in (("w0", W0), ("w1", W1), ("wo", Wout)):
        WT = np.ascontiguousarray(np.asarray(W, np.float32).T)
        com[nm + "a"], com[nm + "l"] = _split_f16(WT)
    for nm, b in (("b0", b0), ("b1", b1), ("b2", bout)):
        hi, lo = _split_f16(np.asarray(b, np.float32))
        com[nm] = np.stack([hi, lo])

    T_ = T
    onesrows = np.zeros((2, T_ * 128), np.float16)
    for t in range(T_):
        onesrows[0, t * 128:(t + 1) * 128] = np.float16(2.0 ** t)
        onesrows[1, t * 128:(t + 1) * 128] = np.float16(2.0 ** (t - 11))
    com["onesrows"] = onesrows
    com["ident"] = np.eye(128, dtype=np.float16)

    in_maps = []
    for c in range(ncores):
        m = dict(com)
        m["xa"] = np.ascontiguousarray(xa[:, :, c * BL:(c + 1) * BL])
        m["xr"] = np.ascontiguousarray(xr[:, :, c * BL:(c + 1) * BL])
        in_maps.append(m)
    return in_maps


def kernel(in_pop_spikes, W0, b0, W1, b1, Wout, bout, batch_size, _trace=False):
    T = in_pop_spikes.shape[2]
    nc = build(**FULL)
    in_maps = prep_inputs(in_pop_spikes, W0, b0, W1, b1, Wout, bout, T=T)
    res = run_bass_kernel_spmd(nc, in_maps, core_ids=list(range(NCORES)),
                               trace=_trace)
    out = np.concatenate([r["out"] for r in res.results], axis=0)
    out = (out / np.float32(T)).astype(np.float32)
    if _trace:
        kernel._last_results = res
    return out
